# revision 1
# baseline (speedup 1.0000x reference)
"""CoGNN layer (GCN message passing + gumbel-hard gating) on 8 trn2 cores.

Sharding: nodes split into 8 contiguous chunks (graph parallel); edges
partitioned by dst core, sorted by dst, bucketed into 128-dst-node blocks
padded to 128-edge subtiles.  Segment sums run on the PE via one-hot
selection matmuls; src-feature gathers use batched indirect DMA from
all-gathered per-node tables.
"""

import os
import sys

sys.path.insert(0, "/opt/trn_rl_repo")

import numpy as np

N, E, D, H, ED = 50000, 800000, 128, 16, 16
NCORES = 8
CHUNK = N // NCORES            # 6250
NBLK = (CHUNK + 127) // 128    # 49
CPAD = NBLK * 128              # 6272
NTAB = NCORES * CPAD           # 50176 rows in all-gathered tables
KB = 1                         # subtile columns per indirect gather call
KB3 = 1                        # for the wide (128-col) z gather
TAU = 0.5

LAST_EXEC_NS = None
LAST_RESULTS = None

_compiled = {}


def _host_prep(x, edge_index, edge_attr, u_in, u_out):
    src = edge_index[0].astype(np.int64)
    dst = edge_index[1].astype(np.int64)
    core = dst // CHUNK
    dloc = dst % CHUNK
    blk = dloc // 128
    lane_dst = (dloc % 128).astype(np.float32)
    src_pad = ((src // CHUNK) * CPAD + (src % CHUNK)).astype(np.int32)

    NG = NCORES * NBLK
    gid = core * NBLK + blk
    order = np.argsort(gid, kind="stable")
    gs = gid[order]
    starts = np.searchsorted(gs, np.arange(NG))
    j = np.arange(E) - starts[gs]

    cnt = np.bincount(gid, minlength=NG).reshape(NCORES, NBLK)
    T = np.maximum(1, -(-cnt.max(0) // 128)).astype(np.int64)   # [NBLK]
    TOFF = np.concatenate([[0], np.cumsum(T)])
    NT = int(TOFF[-1])
    NTP = -(-NT // KB) * KB
    NTP = -(-NTP // KB3) * KB3

    col = TOFF[blk[order]] + j // 128
    lane = j % 128
    core_s = core[order]

    esrcT = np.zeros((NCORES, 128, NTP), np.int32)
    edstT = np.full((NCORES, 128, NTP), -1.0, np.float32)
    eattrT = np.zeros((NCORES, NTP, 16, 128), np.float32)
    esrcT[core_s, lane, col] = src_pad[order]
    edstT[core_s, lane, col] = lane_dst[order]
    eattrT[core_s, col, :, lane] = edge_attr[order]
    eattrT = eattrT.reshape(NCORES, NTP * 16, 128)

    deg1 = (np.bincount(dst, minlength=N) + 1).astype(np.float32)
    deg1_p = np.ones((NCORES, CPAD, 1), np.float32)
    deg1_p[:, :CHUNK, 0] = deg1.reshape(NCORES, CHUNK)

    def pad_nodes(a, fill):
        out = np.full((NCORES, CPAD, a.shape[1]), fill, np.float32)
        out[:, :CHUNK] = a.reshape(NCORES, CHUNK, a.shape[1])
        return out

    return dict(
        T=tuple(int(t) for t in T), NTP=NTP,
        esrcT=esrcT, edstT=edstT, eattrT=eattrT, deg1=deg1_p,
        xs=pad_nodes(x, 0.0),
        u_in=pad_nodes(u_in, 0.5), u_out=pad_nodes(u_out, 0.5),
    )


def _build(T, NTP, use_b1, use_b2, use_epb, use_convb):
    import concourse.bass as bass
    import concourse.bacc as bacc
    import concourse.tile as tile
    from concourse import mybir
    from concourse.bass import IndirectOffsetOnAxis

    f32 = mybir.dt.float32
    i32 = mybir.dt.int32
    AF = mybir.ActivationFunctionType
    OP = mybir.AluOpType

    TOFF = np.concatenate([[0], np.cumsum(T)]).astype(int)
    NT = int(TOFF[-1])

    nc = bacc.Bacc("TRN2", target_bir_lowering=False, debug=False,
                   num_devices=NCORES)
    P = {}
    def param(name, shape, dtype=f32, out=False):
        P[name] = nc.declare_dram_parameter(name, list(shape), dtype, isOutput=out)
        return P[name]

    x_in = param("xs", [CPAD, D])
    esrcT_in = param("esrcT", [128, NTP], i32)
    edstT_in = param("edstT", [128, NTP])
    eattrT_in = param("eattrT", [NTP * 16, 128])
    deg1_in = param("deg1", [CPAD, 1])
    uin_in = param("u_in", [CPAD, 2])
    uout_in = param("u_out", [CPAD, 2])
    w1_in = param("W1cat", [D, 2 * H])
    w2_in = param("W2blk", [2 * H, 4])
    convw_in = param("conv_w", [D, D])
    epw_in = param("ep_w", [ED, D])
    lng_in = param("ln_g", [128, 1])
    lnb_in = param("ln_b", [128, 1])
    b1_in = param("B1REP", [128, 2 * H]) if use_b1 else None
    b2_in = param("B2REP", [128, 4]) if use_b2 else None
    epb_in = param("EPBREP", [128, 128]) if use_epb else None
    convb_in = param("CONVBREP", [128, 128]) if use_convb else None
    iota_in = param("IOTA", [128, 128])
    ident_in = param("IDENT", [128, 128])
    out_p = param("out", [CPAD, D], out=True)

    RG = [list(range(NCORES))]

    with tile.TileContext(nc) as tc:
        with (
            tc.tile_pool(name="singles", bufs=1) as singles,
            tc.tile_pool(name="work", bufs=3) as work,
            tc.tile_pool(name="gath", bufs=3) as gath,
            tc.tile_pool(name="zgath", bufs=3) as zg_pool,
            tc.tile_pool(name="sel", bufs=4) as selp,
            tc.tile_pool(name="acc", bufs=4, space="PSUM") as accp,
            tc.tile_pool(name="psw", bufs=4, space="PSUM") as pswp,
            tc.tile_pool(name="dram", bufs=1, space="DRAM") as dram,
        ):
            # ---- persistent SBUF ----
            def load1(name, ap, shape, dtype=f32):
                t = singles.tile(list(shape), dtype, tag=name)
                nc.sync.dma_start(out=t[:], in_=ap)
                return t

            esrcT = load1("esrcT", esrcT_in[:, :], [128, NTP], i32)
            edstT = load1("edstT", edstT_in[:, :], [128, NTP])
            IOTA = load1("IOTA", iota_in[:, :], [128, 128])
            IDENT = load1("IDENT", ident_in[:, :], [128, 128])
            W1 = load1("W1", w1_in[:, :], [D, 2 * H])
            W2 = load1("W2", w2_in[:, :], [2 * H, 4])
            CONVW = load1("CONVW", convw_in[:, :], [D, D])
            EPW = load1("EPW", epw_in[:, :], [ED, D])
            LNG = load1("LNG", lng_in[:, :], [128, 1])
            LNB = load1("LNB", lnb_in[:, :], [128, 1])
            B1 = load1("B1", b1_in[:, :], [128, 2 * H]) if use_b1 else None
            B2 = load1("B2", b2_in[:, :], [128, 4]) if use_b2 else None
            EPB = load1("EPB", epb_in[:, :], [128, 128]) if use_epb else None
            CONVB = (load1("CONVB", convb_in[:, :], [128, 128])
                     if use_convb else None)
            deg1 = load1(
                "deg1s", deg1_in[:, :].rearrange("(b p) o -> p (b o)", p=128),
                [128, NBLK]
            )

            eps_t = singles.tile([128, 1], f32)
            nc.vector.memset(eps_t[:], 1e-5)
            e10_t = singles.tile([128, 1], f32)
            nc.vector.memset(e10_t[:], 1e-10)

            XW = singles.tile([128, NBLK * D], f32)      # x_norm @ conv_w
            XW1 = singles.tile([128, NBLK * 2 * H], f32)  # x_norm @ W1cat
            RD = singles.tile([128, NBLK * 4], f32)       # r * dinv1^2
            DINV1 = singles.tile([128, NBLK], f32)
            DINV1SQ = singles.tile([128, NBLK], f32)
            INF = singles.tile([128, NBLK], f32)
            OUTF = singles.tile([128, NBLK], f32)
            ASC = singles.tile([128, NBLK], f32)          # in_flag * dinv_w
            DW2 = singles.tile([128, NBLK], f32)          # dinv_w^2

            sq = work.tile([128, NBLK], f32, tag="deg")
            nc.scalar.activation(out=sq[:], in_=deg1[:], func=AF.Sqrt)
            nc.vector.reciprocal(out=DINV1[:], in_=sq[:])
            nc.vector.reciprocal(out=DINV1SQ[:], in_=deg1[:])

            # warm the vector engine's view of late-consumed DMA sems so the
            # per-subtile is_equal ops don't each carry those waits
            warm = work.tile([128, 1], f32, tag="warm")
            nc.vector.tensor_tensor(out=warm[:], in0=edstT[:, 0:1],
                                    in1=IOTA[:, 0:1], op=OP.add)

            # ---- internal DRAM tables ----
            g1loc = dram.tile([CPAD, 2 * H], f32)
            G1 = dram.tile([NTAB, 2 * H], f32)
            qloc = dram.tile([CPAD, 4], f32)
            QT = dram.tile([NTAB, 4], f32)
            ofloc = dram.tile([CPAD, 1], f32)
            OFT = dram.tile([NTAB, 1], f32)
            zloc = dram.tile([CPAD, D], f32)
            ZT = dram.tile([NTAB, D], f32)

            # ================= phase A: layernorm + node matmuls ============
            for b in range(NBLK):
                xt = work.tile([128, D], f32, tag="xt")
                nc.sync.dma_start(out=xt[:], in_=x_in[b * 128:(b + 1) * 128, :])
                stats = work.tile([128, 6], f32, tag="bn")
                nc.vector.bn_stats(out=stats[:], in_=xt[:])
                mv = work.tile([128, 2], f32, tag="mv")
                nc.vector.bn_aggr(out=mv[:], in_=stats[:])
                rstd = work.tile([128, 1], f32, tag="rstd")
                nc.scalar.activation(out=rstd[:], in_=mv[:, 1:2], func=AF.Sqrt,
                                     bias=eps_t[:])
                nc.vector.reciprocal(out=rstd[:], in_=rstd[:])
                y = work.tile([128, D], f32, tag="y")
                nc.vector.tensor_scalar(out=y[:], in0=xt[:],
                                        scalar1=mv[:, 0:1], scalar2=rstd[:],
                                        op0=OP.subtract, op1=OP.mult)
                yT_ps = pswp.tile([128, 128], f32, tag="psw")
                nc.tensor.transpose(out=yT_ps[:], in_=y[:], identity=IDENT[:])
                xnT = work.tile([128, D], f32, tag="xnT")
                nc.vector.tensor_scalar(out=xnT[:], in0=yT_ps[:],
                                        scalar1=LNG[:], scalar2=LNB[:],
                                        op0=OP.mult, op1=OP.add)
                ps1 = pswp.tile([128, 2 * H], f32, tag="psw")
                nc.tensor.matmul(out=ps1[:], lhsT=xnT[:], rhs=W1[:],
                                 start=True, stop=True)
                nc.vector.tensor_copy(out=XW1[:, b * 2 * H:(b + 1) * 2 * H],
                                      in_=ps1[:])
                ps2 = pswp.tile([128, D], f32, tag="psw")
                nc.tensor.matmul(out=ps2[:], lhsT=xnT[:], rhs=CONVW[:],
                                 start=True, stop=True)
                nc.vector.tensor_copy(out=XW[:, b * D:(b + 1) * D], in_=ps2[:])
                g1t = work.tile([128, 2 * H], f32, tag="g1t")
                nc.vector.tensor_scalar_mul(out=g1t[:],
                                            in0=XW1[:, b * 2 * H:(b + 1) * 2 * H],
                                            scalar1=DINV1[:, b:b + 1])
                nc.sync.dma_start(out=g1loc[b * 128:(b + 1) * 128, :], in_=g1t[:])

            nc.gpsimd.collective_compute(
                "AllGather", mybir.AluOpType.bypass, replica_groups=RG,
                ins=[g1loc[:, :]], outs=[G1[:, :]])

            # ================= phase 1: action-net layer 1 ==================
            W1W = 2 * H
            for b in range(NBLK):
                ps = accp.tile([128, W1W], f32, tag="acc")
                for t in range(T[b]):
                    col = int(TOFF[b]) + t
                    g = col // KB
                    if col % KB == 0:
                        gt = gath.tile([128, KB * W1W], f32, tag="g1g")
                        nc.gpsimd.indirect_dma_start(
                            out=gt[:], out_offset=None, in_=G1[:, :],
                            in_offset=IndirectOffsetOnAxis(
                                ap=esrcT[:, g * KB:(g + 1) * KB], axis=0))
                        globals()["_g1buf"] = gt
                    gt = globals()["_g1buf"]
                    sel = selp.tile([128, 128], f32, tag="sel")
                    nc.vector.tensor_tensor(
                        out=sel[:], in0=edstT[:, col:col + 1].to_broadcast([128, 128]),
                        in1=IOTA[:], op=OP.is_equal)
                    cc = col % KB
                    nc.tensor.matmul(out=ps[:], lhsT=sel[:],
                                     rhs=gt[:, cc * W1W:(cc + 1) * W1W],
                                     start=(t == 0), stop=(t == T[b] - 1))
                t1 = work.tile([128, W1W], f32, tag="hpre")
                nc.vector.tensor_scalar_mul(out=t1[:], in0=ps[:],
                                            scalar1=DINV1[:, b:b + 1])
                t2 = work.tile([128, W1W], f32, tag="hpre2")
                nc.vector.tensor_scalar_mul(
                    out=t2[:], in0=XW1[:, b * W1W:(b + 1) * W1W],
                    scalar1=DINV1SQ[:, b:b + 1])
                nc.vector.tensor_add(out=t1[:], in0=t1[:], in1=t2[:])
                if use_b1:
                    nc.vector.tensor_add(out=t1[:], in0=t1[:], in1=B1[:])
                h = work.tile([128, W1W], f32, tag="h")
                nc.scalar.activation(out=h[:], in_=t1[:], func=AF.Relu)
                hT_ps = pswp.tile([128, 128], f32, tag="psw")
                nc.tensor.transpose(out=hT_ps[:W1W, :], in_=h[:], identity=IDENT[:])
                hT = work.tile([W1W, 128], f32, tag="hT")
                nc.vector.tensor_copy(out=hT[:], in_=hT_ps[:W1W, :])
                r_ps = pswp.tile([128, 4], f32, tag="psw")
                nc.tensor.matmul(out=r_ps[:], lhsT=hT[:], rhs=W2[:],
                                 start=True, stop=True)
                qt = work.tile([128, 4], f32, tag="qt")
                nc.vector.tensor_scalar_mul(out=qt[:], in0=r_ps[:],
                                            scalar1=DINV1[:, b:b + 1])
                nc.sync.dma_start(out=qloc[b * 128:(b + 1) * 128, :], in_=qt[:])
                nc.vector.tensor_scalar_mul(out=RD[:, b * 4:(b + 1) * 4],
                                            in0=r_ps[:],
                                            scalar1=DINV1SQ[:, b:b + 1])

            nc.gpsimd.collective_compute(
                "AllGather", mybir.AluOpType.bypass, replica_groups=RG,
                ins=[qloc[:, :]], outs=[QT[:, :]])

            # ================= phase 2: action-net layer 2 + flags ==========
            for b in range(NBLK):
                ps = accp.tile([128, 4], f32, tag="acc")
                for t in range(T[b]):
                    col = int(TOFF[b]) + t
                    g = col // KB
                    if col % KB == 0:
                        gt = gath.tile([128, KB * 4], f32, tag="g2g")
                        nc.gpsimd.indirect_dma_start(
                            out=gt[:], out_offset=None, in_=QT[:, :],
                            in_offset=IndirectOffsetOnAxis(
                                ap=esrcT[:, g * KB:(g + 1) * KB], axis=0))
                        globals()["_g2buf"] = gt
                    gt = globals()["_g2buf"]
                    sel = selp.tile([128, 128], f32, tag="sel")
                    nc.vector.tensor_tensor(
                        out=sel[:], in0=edstT[:, col:col + 1].to_broadcast([128, 128]),
                        in1=IOTA[:], op=OP.is_equal)
                    cc = col % KB
                    nc.tensor.matmul(out=ps[:], lhsT=sel[:],
                                     rhs=gt[:, cc * 4:(cc + 1) * 4],
                                     start=(t == 0), stop=(t == T[b] - 1))
                lg = work.tile([128, 4], f32, tag="lg")
                nc.vector.tensor_scalar_mul(out=lg[:], in0=ps[:],
                                            scalar1=DINV1[:, b:b + 1])
                nc.vector.tensor_add(out=lg[:], in0=lg[:],
                                     in1=RD[:, b * 4:(b + 1) * 4])
                if use_b2:
                    nc.vector.tensor_add(out=lg[:], in0=lg[:], in1=B2[:])
                for name, u_ap, fl, c0 in (
                    ("in", uin_in, INF, 0), ("out", uout_in, OUTF, 2),
                ):
                    ut = work.tile([128, 2], f32, tag="ut")
                    nc.sync.dma_start(out=ut[:],
                                      in_=u_ap[b * 128:(b + 1) * 128, :])
                    lnu = work.tile([128, 2], f32, tag="lnu")
                    nc.scalar.activation(out=lnu[:], in_=ut[:], func=AF.Ln,
                                         bias=e10_t[:])
                    lw = work.tile([128, 2], f32, tag="lw")
                    nc.scalar.activation(out=lw[:], in_=lnu[:], func=AF.Ln,
                                         bias=e10_t[:], scale=-1.0)
                    a0 = work.tile([128, 1], f32, tag="a0")
                    nc.vector.tensor_add(out=a0[:], in0=lg[:, c0:c0 + 1],
                                         in1=lw[:, 1:2])
                    a1 = work.tile([128, 1], f32, tag="a1")
                    nc.vector.tensor_add(out=a1[:], in0=lg[:, c0 + 1:c0 + 2],
                                         in1=lw[:, 0:1])
                    nc.vector.tensor_tensor(out=fl[:, b:b + 1], in0=a0[:],
                                            in1=a1[:], op=OP.is_ge)
                oft = work.tile([128, 1], f32, tag="oft")
                nc.vector.tensor_copy(out=oft[:], in_=OUTF[:, b:b + 1])
                nc.sync.dma_start(out=ofloc[b * 128:(b + 1) * 128, :], in_=oft[:])

            nc.gpsimd.collective_compute(
                "AllGather", mybir.AluOpType.bypass, replica_groups=RG,
                ins=[ofloc[:, :]], outs=[OFT[:, :]])

            # ================= phase S: deg_w from out-flags ================
            for b in range(NBLK):
                ps = accp.tile([128, 1], f32, tag="acc")
                for t in range(T[b]):
                    col = int(TOFF[b]) + t
                    g = col // KB
                    if col % KB == 0:
                        gt = gath.tile([128, KB], f32, tag="gSg")
                        nc.gpsimd.indirect_dma_start(
                            out=gt[:], out_offset=None, in_=OFT[:, :],
                            in_offset=IndirectOffsetOnAxis(
                                ap=esrcT[:, g * KB:(g + 1) * KB], axis=0))
                        globals()["_gSbuf"] = gt
                    gt = globals()["_gSbuf"]
                    sel = selp.tile([128, 128], f32, tag="sel")
                    nc.vector.tensor_tensor(
                        out=sel[:], in0=edstT[:, col:col + 1].to_broadcast([128, 128]),
                        in1=IOTA[:], op=OP.is_equal)
                    cc = col % KB
                    nc.tensor.matmul(out=ps[:], lhsT=sel[:],
                                     rhs=gt[:, cc:cc + 1],
                                     start=(t == 0), stop=(t == T[b] - 1))
                dw = work.tile([128, 1], f32, tag="dw")
                nc.vector.tensor_scalar_mul(out=dw[:], in0=ps[:],
                                            scalar1=INF[:, b:b + 1])
                nc.vector.tensor_scalar_add(out=dw[:], in0=dw[:], scalar1=1.0)
                sq2 = work.tile([128, 1], f32, tag="sq2")
                nc.scalar.activation(out=sq2[:], in_=dw[:], func=AF.Sqrt)
                dwi = work.tile([128, 1], f32, tag="dwi")
                nc.vector.reciprocal(out=dwi[:], in_=sq2[:])
                nc.vector.reciprocal(out=DW2[:, b:b + 1], in_=dw[:])
                nc.vector.tensor_mul(out=ASC[:, b:b + 1], in0=INF[:, b:b + 1],
                                     in1=dwi[:])
                tv = work.tile([128, 1], f32, tag="tv")
                nc.vector.tensor_mul(out=tv[:], in0=OUTF[:, b:b + 1], in1=dwi[:])
                zt = work.tile([128, D], f32, tag="zt")
                nc.vector.tensor_scalar_mul(out=zt[:],
                                            in0=XW[:, b * D:(b + 1) * D],
                                            scalar1=tv[:])
                nc.sync.dma_start(out=zloc[b * 128:(b + 1) * 128, :], in_=zt[:])

            nc.gpsimd.collective_compute(
                "AllGather", mybir.AluOpType.bypass, replica_groups=RG,
                ins=[zloc[:, :]], outs=[ZT[:, :]])

            # ================= phase 3: main conv + edge features ===========
            for b in range(NBLK):
                psA = accp.tile([128, D], f32, tag="acc")
                psB = accp.tile([128, D], f32, tag="acc")
                for t in range(T[b]):
                    col = int(TOFF[b]) + t
                    g = col // KB3
                    if col % KB3 == 0:
                        gt = zg_pool.tile([128, KB3 * D], f32, tag="zgg")
                        nc.gpsimd.indirect_dma_start(
                            out=gt[:], out_offset=None, in_=ZT[:, :],
                            in_offset=IndirectOffsetOnAxis(
                                ap=esrcT[:, g * KB3:(g + 1) * KB3], axis=0))
                        globals()["_zbuf"] = gt
                    gt = globals()["_zbuf"]
                    eaT = work.tile([ED, 128], f32, tag="eaT")
                    nc.sync.dma_start(out=eaT[:],
                                      in_=eattrT_in[col * ED:(col + 1) * ED, :])
                    ef_ps = pswp.tile([128, D], f32, tag="psw")
                    nc.tensor.matmul(out=ef_ps[:], lhsT=eaT[:], rhs=EPW[:],
                                     start=True, stop=True)
                    ef = work.tile([128, D], f32, tag="ef")
                    if use_epb:
                        nc.vector.tensor_add(out=ef_ps[:], in0=ef_ps[:], in1=EPB[:])
                    nc.scalar.activation(out=ef[:], in_=ef_ps[:], func=AF.Relu)
                    sel = selp.tile([128, 128], f32, tag="sel")
                    nc.vector.tensor_tensor(
                        out=sel[:], in0=edstT[:, col:col + 1].to_broadcast([128, 128]),
                        in1=IOTA[:], op=OP.is_equal)
                    cc = col % KB3
                    nc.tensor.matmul(out=psA[:], lhsT=sel[:],
                                     rhs=gt[:, cc * D:(cc + 1) * D],
                                     start=(t == 0), stop=(t == T[b] - 1))
                    nc.tensor.matmul(out=psB[:], lhsT=sel[:], rhs=ef[:],
                                     start=(t == 0), stop=(t == T[b] - 1))
                o1 = work.tile([128, D], f32, tag="o1")
                nc.vector.tensor_scalar_mul(out=o1[:], in0=psA[:],
                                            scalar1=ASC[:, b:b + 1])
                o2 = work.tile([128, D], f32, tag="o2")
                nc.vector.tensor_scalar_mul(out=o2[:],
                                            in0=XW[:, b * D:(b + 1) * D],
                                            scalar1=DW2[:, b:b + 1])
                nc.vector.tensor_add(out=o1[:], in0=o1[:], in1=psB[:])
                nc.vector.tensor_add(out=o1[:], in0=o1[:], in1=o2[:])
                if use_convb:
                    nc.vector.tensor_add(out=o1[:], in0=o1[:], in1=CONVB[:])
                ot = work.tile([128, D], f32, tag="ot")
                nc.scalar.activation(out=ot[:], in_=o1[:], func=AF.Relu)
                nc.sync.dma_start(out=out_p[b * 128:(b + 1) * 128, :], in_=ot[:])

    nc.compile()
    return nc


def kernel(x, edge_index, edge_attr, u_in, u_out, ln_g, ln_b, conv_w, conv_b,
           ep_w, ep_b, ia1_w, ia1_b, ia2_w, ia2_b, oa1_w, oa1_b, oa2_w, oa2_b):
    global LAST_EXEC_NS, LAST_RESULTS
    from concourse.bass_utils import run_bass_kernel_spmd

    x = np.asarray(x, np.float32)
    edge_index = np.asarray(edge_index, np.int32)
    edge_attr = np.asarray(edge_attr, np.float32)
    u_in = np.asarray(u_in, np.float32)
    u_out = np.asarray(u_out, np.float32)

    prep = _host_prep(x, edge_index, edge_attr, u_in, u_out)

    W1cat = np.concatenate([ia1_w, oa1_w], axis=1).astype(np.float32)
    W2blk = np.zeros((2 * H, 4), np.float32)
    W2blk[:H, :2] = ia2_w
    W2blk[H:, 2:] = oa2_w
    b1cat = np.concatenate([ia1_b, oa1_b]).astype(np.float32)
    b2cat = np.concatenate([ia2_b, oa2_b]).astype(np.float32)
    use_b1 = bool(np.any(b1cat))
    use_b2 = bool(np.any(b2cat))
    use_epb = bool(np.any(np.asarray(ep_b)))
    use_convb = bool(np.any(np.asarray(conv_b)))

    key = (prep["T"], prep["NTP"], use_b1, use_b2, use_epb, use_convb)
    if key not in _compiled:
        _compiled[key] = _build(prep["T"], prep["NTP"],
                                use_b1, use_b2, use_epb, use_convb)
    nc = _compiled[key]

    common = dict(
        W1cat=W1cat, W2blk=W2blk,
        conv_w=np.asarray(conv_w, np.float32),
        ep_w=np.asarray(ep_w, np.float32),
        ln_g=np.asarray(ln_g, np.float32).reshape(128, 1),
        ln_b=np.asarray(ln_b, np.float32).reshape(128, 1),
        IOTA=np.tile(np.arange(128, dtype=np.float32), (128, 1)),
        IDENT=np.eye(128, dtype=np.float32),
    )
    if use_b1:
        common["B1REP"] = np.tile(b1cat, (128, 1))
    if use_b2:
        common["B2REP"] = np.tile(b2cat, (128, 1))
    if use_epb:
        common["EPBREP"] = np.tile(np.asarray(ep_b, np.float32), (128, 1))
    if use_convb:
        common["CONVBREP"] = np.tile(np.asarray(conv_b, np.float32), (128, 1))

    in_maps = []
    for c in range(NCORES):
        m = dict(common)
        m["xs"] = prep["xs"][c]
        m["esrcT"] = prep["esrcT"][c]
        m["edstT"] = prep["edstT"][c]
        m["eattrT"] = prep["eattrT"][c]
        m["deg1"] = prep["deg1"][c]
        m["u_in"] = prep["u_in"][c]
        m["u_out"] = prep["u_out"][c]
        in_maps.append(m)

    import time as _time
    trace = bool(os.environ.get("KERNEL_TRACE"))
    t0 = _time.time()
    try:
        res = run_bass_kernel_spmd(nc, in_maps, list(range(NCORES)), trace=trace)
    except ModuleNotFoundError:
        res = run_bass_kernel_spmd(nc, in_maps, list(range(NCORES)), trace=False)
    globals()["LAST_WALL_NS"] = int((_time.time() - t0) * 1e9)
    LAST_EXEC_NS = res.exec_time_ns
    LAST_RESULTS = res

    out = np.empty((N, D), np.float32)
    for c in range(NCORES):
        out[c * CHUNK:(c + 1) * CHUNK] = res.results[c]["out"][:CHUNK]
    return out



# revision 2
# speedup vs baseline: 39.9745x; 39.9745x over previous
"""CoGNN layer (GCN message passing + gumbel-hard gating) on 8 trn2 cores.

Sharding: nodes split into 8 contiguous chunks (graph parallel); edges
partitioned by dst core, sorted by dst, bucketed into 128-dst-node blocks
padded to 128-edge subtiles.  Segment sums run on the PE via one-hot
selection matmuls; src-feature gathers use batched indirect DMA from
all-gathered per-node tables.
"""

import os
import sys

sys.path.insert(0, "/opt/trn_rl_repo")

import numpy as np

N, E, D, H, ED = 50000, 800000, 128, 16, 16
NCORES = 8
CHUNK = N // NCORES            # 6250
NBLK = (CHUNK + 127) // 128    # 49
CPAD = NBLK * 128              # 6272
NTAB = NCORES * CPAD           # 50176 rows in all-gathered tables
KB = 1                         # subtile columns per indirect gather call
KB3 = 1                        # for the wide (128-col) z gather
TAU = 0.5

LAST_EXEC_NS = None
LAST_RESULTS = None

_compiled = {}


def _host_prep(x, edge_index, edge_attr, u_in, u_out):
    src = edge_index[0].astype(np.int64)
    dst = edge_index[1].astype(np.int64)
    core = dst // CHUNK
    dloc = dst % CHUNK
    blk = dloc // 128
    lane_dst = (dloc % 128).astype(np.float32)
    src_pad = ((src // CHUNK) * CPAD + (src % CHUNK)).astype(np.int32)

    NG = NCORES * NBLK
    gid = core * NBLK + blk
    order = np.argsort(gid, kind="stable")
    gs = gid[order]
    starts = np.searchsorted(gs, np.arange(NG))
    j = np.arange(E) - starts[gs]

    cnt = np.bincount(gid, minlength=NG).reshape(NCORES, NBLK)
    T = np.maximum(1, -(-cnt.max(0) // 128)).astype(np.int64)   # [NBLK]
    TOFF = np.concatenate([[0], np.cumsum(T)])
    NT = int(TOFF[-1])
    NTP = -(-NT // KB) * KB
    NTP = -(-NTP // KB3) * KB3

    col = TOFF[blk[order]] + j // 128
    lane = j % 128
    core_s = core[order]

    esrcT = np.zeros((NCORES, 128, NTP), np.int32)
    edstT = np.full((NCORES, 128, NTP), -1.0, np.float32)
    eattrT = np.zeros((NCORES, NTP, 16, 128), np.float32)
    esrcT[core_s, lane, col] = src_pad[order]
    edstT[core_s, lane, col] = lane_dst[order]
    eattrT[core_s, col, :, lane] = edge_attr[order]
    eattrT = eattrT.reshape(NCORES, NTP * 16, 128)

    deg1 = (np.bincount(dst, minlength=N) + 1).astype(np.float32)
    deg1_p = np.ones((NCORES, CPAD, 1), np.float32)
    deg1_p[:, :CHUNK, 0] = deg1.reshape(NCORES, CHUNK)

    def pad_nodes(a, fill):
        out = np.full((NCORES, CPAD, a.shape[1]), fill, np.float32)
        out[:, :CHUNK] = a.reshape(NCORES, CHUNK, a.shape[1])
        return out

    return dict(
        T=tuple(int(t) for t in T), NTP=NTP,
        esrcT=esrcT, edstT=edstT, eattrT=eattrT, deg1=deg1_p,
        xs=pad_nodes(x, 0.0),
        u_in=pad_nodes(u_in, 0.5), u_out=pad_nodes(u_out, 0.5),
    )


def _build(T, NTP, use_b1, use_b2, use_epb, use_convb):
    import concourse.bass as bass
    import concourse.bacc as bacc
    import concourse.tile as tile
    from concourse import mybir
    from concourse.bass import IndirectOffsetOnAxis

    f32 = mybir.dt.float32
    i32 = mybir.dt.int32
    AF = mybir.ActivationFunctionType
    OP = mybir.AluOpType

    TOFF = np.concatenate([[0], np.cumsum(T)]).astype(int)
    NT = int(TOFF[-1])

    nc = bacc.Bacc("TRN2", target_bir_lowering=False, debug=False,
                   num_devices=NCORES)
    P = {}
    def param(name, shape, dtype=f32, out=False):
        P[name] = nc.declare_dram_parameter(name, list(shape), dtype, isOutput=out)
        return P[name]

    x_in = param("xs", [CPAD, D])
    esrcT_in = param("esrcT", [128, NTP], i32)
    edstT_in = param("edstT", [128, NTP])
    eattrT_in = param("eattrT", [NTP * 16, 128])
    deg1_in = param("deg1", [CPAD, 1])
    uin_in = param("u_in", [CPAD, 2])
    uout_in = param("u_out", [CPAD, 2])
    w1_in = param("W1cat", [D, 2 * H])
    w2_in = param("W2blk", [2 * H, 4])
    convw_in = param("conv_w", [D, D])
    epw_in = param("ep_w", [ED, D])
    lng_in = param("ln_g", [128, 1])
    lnb_in = param("ln_b", [128, 1])
    b1_in = param("B1REP", [128, 2 * H]) if use_b1 else None
    b2_in = param("B2REP", [128, 4]) if use_b2 else None
    epb_in = param("EPBREP", [128, 128]) if use_epb else None
    convb_in = param("CONVBREP", [128, 128]) if use_convb else None
    iota_in = param("IOTA", [128, 128])
    ident_in = param("IDENT", [128, 128])
    out_p = param("out", [CPAD, D], out=True)

    RG = [list(range(NCORES))]

    with tile.TileContext(nc) as tc:
        with (
            tc.tile_pool(name="singles", bufs=1) as singles,
            tc.tile_pool(name="work", bufs=3) as work,
            tc.tile_pool(name="gath", bufs=3) as gath,
            tc.tile_pool(name="zgath", bufs=3) as zg_pool,
            tc.tile_pool(name="sel", bufs=4) as selp,
            tc.tile_pool(name="acc", bufs=4, space="PSUM") as accp,
            tc.tile_pool(name="psw", bufs=4, space="PSUM") as pswp,
            tc.tile_pool(name="dram", bufs=1, space="DRAM") as dram,
        ):
            # ---- persistent SBUF ----
            def load1(name, ap, shape, dtype=f32):
                t = singles.tile(list(shape), dtype, tag=name)
                nc.sync.dma_start(out=t[:], in_=ap)
                return t

            esrcT = load1("esrcT", esrcT_in[:, :], [128, NTP], i32)
            edstT = load1("edstT", edstT_in[:, :], [128, NTP])
            IOTA = load1("IOTA", iota_in[:, :], [128, 128])
            IDENT = load1("IDENT", ident_in[:, :], [128, 128])
            W1 = load1("W1", w1_in[:, :], [D, 2 * H])
            W2 = load1("W2", w2_in[:, :], [2 * H, 4])
            CONVW = load1("CONVW", convw_in[:, :], [D, D])
            EPW = load1("EPW", epw_in[:, :], [ED, D])
            LNG = load1("LNG", lng_in[:, :], [128, 1])
            LNB = load1("LNB", lnb_in[:, :], [128, 1])
            B1 = load1("B1", b1_in[:, :], [128, 2 * H]) if use_b1 else None
            B2 = load1("B2", b2_in[:, :], [128, 4]) if use_b2 else None
            EPB = load1("EPB", epb_in[:, :], [128, 128]) if use_epb else None
            CONVB = (load1("CONVB", convb_in[:, :], [128, 128])
                     if use_convb else None)
            deg1 = load1(
                "deg1s", deg1_in[:, :].rearrange("(b p) o -> p (b o)", p=128),
                [128, NBLK]
            )

            eps_t = singles.tile([128, 1], f32)
            nc.vector.memset(eps_t[:], 1e-5)
            e10_t = singles.tile([128, 1], f32)
            nc.vector.memset(e10_t[:], 1e-10)

            XW = singles.tile([128, NBLK * D], f32)      # x_norm @ conv_w
            XW1 = singles.tile([128, NBLK * 2 * H], f32)  # x_norm @ W1cat
            RD = singles.tile([128, NBLK * 4], f32)       # r * dinv1^2
            DINV1 = singles.tile([128, NBLK], f32)
            DINV1SQ = singles.tile([128, NBLK], f32)
            INF = singles.tile([128, NBLK], f32)
            OUTF = singles.tile([128, NBLK], f32)
            ASC = singles.tile([128, NBLK], f32)          # in_flag * dinv_w
            DW2 = singles.tile([128, NBLK], f32)          # dinv_w^2

            sq = work.tile([128, NBLK], f32, tag="deg")
            nc.scalar.activation(out=sq[:], in_=deg1[:], func=AF.Sqrt)
            nc.vector.reciprocal(out=DINV1[:], in_=sq[:])
            nc.vector.reciprocal(out=DINV1SQ[:], in_=deg1[:])

            # warm the vector engine's view of late-consumed DMA sems so the
            # per-subtile is_equal ops don't each carry those waits
            warm = work.tile([128, 1], f32, tag="warm")
            nc.vector.tensor_tensor(out=warm[:], in0=edstT[:, 0:1],
                                    in1=IOTA[:, 0:1], op=OP.add)

            # ---- internal DRAM tables ----
            bf16 = mybir.dt.bfloat16
            g1loc = dram.tile([CPAD, 2 * H], f32)
            G1 = nc.dram_tensor("G1S", [NTAB, 2 * H], f32, kind="Internal",
                                addr_space="Shared")
            qloc = dram.tile([CPAD, 4], f32)
            QT = nc.dram_tensor("QTS", [NTAB, 4], f32, kind="Internal",
                                addr_space="Shared")
            ofloc = dram.tile([CPAD, 1], f32)
            OFT = nc.dram_tensor("OFTS", [NTAB, 1], f32, kind="Internal",
                                 addr_space="Shared")
            zloc = dram.tile([CPAD, D], bf16)
            ZT = nc.dram_tensor("ZTS", [NTAB, D], bf16, kind="Internal",
                                addr_space="Shared")

            # ================= phase A: layernorm + node matmuls ============
            for b in range(NBLK):
                xt = work.tile([128, D], f32, tag="xt")
                nc.sync.dma_start(out=xt[:], in_=x_in[b * 128:(b + 1) * 128, :])
                stats = work.tile([128, 6], f32, tag="bn")
                nc.vector.bn_stats(out=stats[:], in_=xt[:])
                mv = work.tile([128, 2], f32, tag="mv")
                nc.vector.bn_aggr(out=mv[:], in_=stats[:])
                rstd = work.tile([128, 1], f32, tag="rstd")
                nc.scalar.activation(out=rstd[:], in_=mv[:, 1:2], func=AF.Sqrt,
                                     bias=eps_t[:])
                nc.vector.reciprocal(out=rstd[:], in_=rstd[:])
                y = work.tile([128, D], f32, tag="y")
                nc.vector.tensor_scalar(out=y[:], in0=xt[:],
                                        scalar1=mv[:, 0:1], scalar2=rstd[:],
                                        op0=OP.subtract, op1=OP.mult)
                yT_ps = pswp.tile([128, 128], f32, tag="psw")
                nc.tensor.transpose(out=yT_ps[:], in_=y[:], identity=IDENT[:])
                xnT = work.tile([128, D], f32, tag="xnT")
                nc.vector.tensor_scalar(out=xnT[:], in0=yT_ps[:],
                                        scalar1=LNG[:], scalar2=LNB[:],
                                        op0=OP.mult, op1=OP.add)
                ps1 = pswp.tile([128, 2 * H], f32, tag="psw")
                nc.tensor.matmul(out=ps1[:], lhsT=xnT[:], rhs=W1[:],
                                 start=True, stop=True)
                nc.vector.tensor_copy(out=XW1[:, b * 2 * H:(b + 1) * 2 * H],
                                      in_=ps1[:])
                ps2 = pswp.tile([128, D], f32, tag="psw")
                nc.tensor.matmul(out=ps2[:], lhsT=xnT[:], rhs=CONVW[:],
                                 start=True, stop=True)
                nc.vector.tensor_copy(out=XW[:, b * D:(b + 1) * D], in_=ps2[:])
                g1t = work.tile([128, 2 * H], f32, tag="g1t")
                nc.vector.tensor_scalar_mul(out=g1t[:],
                                            in0=XW1[:, b * 2 * H:(b + 1) * 2 * H],
                                            scalar1=DINV1[:, b:b + 1])
                nc.sync.dma_start(out=g1loc[b * 128:(b + 1) * 128, :], in_=g1t[:])

            nc.gpsimd.collective_compute(
                "AllGather", mybir.AluOpType.bypass, replica_groups=RG,
                ins=[g1loc[:, :]], outs=[G1[:, :]])

            # ================= phase 1: action-net layer 1 ==================
            W1W = 2 * H
            for b in range(NBLK):
                ps = accp.tile([128, W1W], f32, tag="acc")
                for t in range(T[b]):
                    col = int(TOFF[b]) + t
                    g = col // KB
                    if col % KB == 0:
                        gt = gath.tile([128, KB * W1W], f32, tag="g1g")
                        nc.gpsimd.indirect_dma_start(
                            out=gt[:], out_offset=None, in_=G1[:, :],
                            in_offset=IndirectOffsetOnAxis(
                                ap=esrcT[:, g * KB:(g + 1) * KB], axis=0))
                        globals()["_g1buf"] = gt
                    gt = globals()["_g1buf"]
                    sel = selp.tile([128, 128], f32, tag="sel")
                    nc.vector.tensor_tensor(
                        out=sel[:], in0=edstT[:, col:col + 1].to_broadcast([128, 128]),
                        in1=IOTA[:], op=OP.is_equal)
                    cc = col % KB
                    nc.tensor.matmul(out=ps[:], lhsT=sel[:],
                                     rhs=gt[:, cc * W1W:(cc + 1) * W1W],
                                     start=(t == 0), stop=(t == T[b] - 1))
                t1 = work.tile([128, W1W], f32, tag="hpre")
                nc.vector.tensor_scalar_mul(out=t1[:], in0=ps[:],
                                            scalar1=DINV1[:, b:b + 1])
                t2 = work.tile([128, W1W], f32, tag="hpre2")
                nc.vector.tensor_scalar_mul(
                    out=t2[:], in0=XW1[:, b * W1W:(b + 1) * W1W],
                    scalar1=DINV1SQ[:, b:b + 1])
                nc.vector.tensor_add(out=t1[:], in0=t1[:], in1=t2[:])
                if use_b1:
                    nc.vector.tensor_add(out=t1[:], in0=t1[:], in1=B1[:])
                h = work.tile([128, W1W], f32, tag="h")
                nc.scalar.activation(out=h[:], in_=t1[:], func=AF.Relu)
                hT_ps = pswp.tile([128, 128], f32, tag="psw")
                nc.tensor.transpose(out=hT_ps[:W1W, :], in_=h[:], identity=IDENT[:])
                hT = work.tile([W1W, 128], f32, tag="hT")
                nc.vector.tensor_copy(out=hT[:], in_=hT_ps[:W1W, :])
                r_ps = pswp.tile([128, 4], f32, tag="psw")
                nc.tensor.matmul(out=r_ps[:], lhsT=hT[:], rhs=W2[:],
                                 start=True, stop=True)
                qt = work.tile([128, 4], f32, tag="qt")
                nc.vector.tensor_scalar_mul(out=qt[:], in0=r_ps[:],
                                            scalar1=DINV1[:, b:b + 1])
                nc.sync.dma_start(out=qloc[b * 128:(b + 1) * 128, :], in_=qt[:])
                nc.vector.tensor_scalar_mul(out=RD[:, b * 4:(b + 1) * 4],
                                            in0=r_ps[:],
                                            scalar1=DINV1SQ[:, b:b + 1])

            nc.gpsimd.collective_compute(
                "AllGather", mybir.AluOpType.bypass, replica_groups=RG,
                ins=[qloc[:, :]], outs=[QT[:, :]])

            # ================= phase 2: action-net layer 2 + flags ==========
            for b in range(NBLK):
                ps = accp.tile([128, 4], f32, tag="acc")
                for t in range(T[b]):
                    col = int(TOFF[b]) + t
                    g = col // KB
                    if col % KB == 0:
                        gt = gath.tile([128, KB * 4], f32, tag="g2g")
                        nc.gpsimd.indirect_dma_start(
                            out=gt[:], out_offset=None, in_=QT[:, :],
                            in_offset=IndirectOffsetOnAxis(
                                ap=esrcT[:, g * KB:(g + 1) * KB], axis=0))
                        globals()["_g2buf"] = gt
                    gt = globals()["_g2buf"]
                    sel = selp.tile([128, 128], f32, tag="sel")
                    nc.vector.tensor_tensor(
                        out=sel[:], in0=edstT[:, col:col + 1].to_broadcast([128, 128]),
                        in1=IOTA[:], op=OP.is_equal)
                    cc = col % KB
                    nc.tensor.matmul(out=ps[:], lhsT=sel[:],
                                     rhs=gt[:, cc * 4:(cc + 1) * 4],
                                     start=(t == 0), stop=(t == T[b] - 1))
                lg = work.tile([128, 4], f32, tag="lg")
                nc.vector.tensor_scalar_mul(out=lg[:], in0=ps[:],
                                            scalar1=DINV1[:, b:b + 1])
                nc.vector.tensor_add(out=lg[:], in0=lg[:],
                                     in1=RD[:, b * 4:(b + 1) * 4])
                if use_b2:
                    nc.vector.tensor_add(out=lg[:], in0=lg[:], in1=B2[:])
                for name, u_ap, fl, c0 in (
                    ("in", uin_in, INF, 0), ("out", uout_in, OUTF, 2),
                ):
                    ut = work.tile([128, 2], f32, tag="ut")
                    nc.sync.dma_start(out=ut[:],
                                      in_=u_ap[b * 128:(b + 1) * 128, :])
                    lnu = work.tile([128, 2], f32, tag="lnu")
                    nc.scalar.activation(out=lnu[:], in_=ut[:], func=AF.Ln,
                                         bias=e10_t[:])
                    lw = work.tile([128, 2], f32, tag="lw")
                    nc.scalar.activation(out=lw[:], in_=lnu[:], func=AF.Ln,
                                         bias=e10_t[:], scale=-1.0)
                    a0 = work.tile([128, 1], f32, tag="a0")
                    nc.vector.tensor_add(out=a0[:], in0=lg[:, c0:c0 + 1],
                                         in1=lw[:, 1:2])
                    a1 = work.tile([128, 1], f32, tag="a1")
                    nc.vector.tensor_add(out=a1[:], in0=lg[:, c0 + 1:c0 + 2],
                                         in1=lw[:, 0:1])
                    nc.vector.tensor_tensor(out=fl[:, b:b + 1], in0=a0[:],
                                            in1=a1[:], op=OP.is_ge)
                oft = work.tile([128, 1], f32, tag="oft")
                nc.vector.tensor_copy(out=oft[:], in_=OUTF[:, b:b + 1])
                nc.sync.dma_start(out=ofloc[b * 128:(b + 1) * 128, :], in_=oft[:])

            nc.gpsimd.collective_compute(
                "AllGather", mybir.AluOpType.bypass, replica_groups=RG,
                ins=[ofloc[:, :]], outs=[OFT[:, :]])

            # ================= phase S: deg_w from out-flags ================
            for b in range(NBLK):
                ps = accp.tile([128, 1], f32, tag="acc")
                for t in range(T[b]):
                    col = int(TOFF[b]) + t
                    g = col // KB
                    if col % KB == 0:
                        gt = gath.tile([128, KB], f32, tag="gSg")
                        nc.gpsimd.indirect_dma_start(
                            out=gt[:], out_offset=None, in_=OFT[:, :],
                            in_offset=IndirectOffsetOnAxis(
                                ap=esrcT[:, g * KB:(g + 1) * KB], axis=0))
                        globals()["_gSbuf"] = gt
                    gt = globals()["_gSbuf"]
                    sel = selp.tile([128, 128], f32, tag="sel")
                    nc.vector.tensor_tensor(
                        out=sel[:], in0=edstT[:, col:col + 1].to_broadcast([128, 128]),
                        in1=IOTA[:], op=OP.is_equal)
                    cc = col % KB
                    nc.tensor.matmul(out=ps[:], lhsT=sel[:],
                                     rhs=gt[:, cc:cc + 1],
                                     start=(t == 0), stop=(t == T[b] - 1))
                dw = work.tile([128, 1], f32, tag="dw")
                nc.vector.tensor_scalar_mul(out=dw[:], in0=ps[:],
                                            scalar1=INF[:, b:b + 1])
                nc.vector.tensor_scalar_add(out=dw[:], in0=dw[:], scalar1=1.0)
                sq2 = work.tile([128, 1], f32, tag="sq2")
                nc.scalar.activation(out=sq2[:], in_=dw[:], func=AF.Sqrt)
                dwi = work.tile([128, 1], f32, tag="dwi")
                nc.vector.reciprocal(out=dwi[:], in_=sq2[:])
                nc.vector.reciprocal(out=DW2[:, b:b + 1], in_=dw[:])
                nc.vector.tensor_mul(out=ASC[:, b:b + 1], in0=INF[:, b:b + 1],
                                     in1=dwi[:])
                tv = work.tile([128, 1], f32, tag="tv")
                nc.vector.tensor_mul(out=tv[:], in0=OUTF[:, b:b + 1], in1=dwi[:])
                zt = work.tile([128, D], bf16, tag="zt")
                nc.vector.tensor_scalar_mul(out=zt[:],
                                            in0=XW[:, b * D:(b + 1) * D],
                                            scalar1=tv[:])
                nc.sync.dma_start(out=zloc[b * 128:(b + 1) * 128, :], in_=zt[:])

            nc.gpsimd.collective_compute(
                "AllGather", mybir.AluOpType.bypass, replica_groups=RG,
                ins=[zloc[:, :]], outs=[ZT[:, :]])

            # ================= phase 3: main conv + edge features ===========
            for b in range(NBLK):
                psA = accp.tile([128, D], f32, tag="acc")
                psB = accp.tile([128, D], f32, tag="acc")
                for t in range(T[b]):
                    col = int(TOFF[b]) + t
                    g = col // KB3
                    if col % KB3 == 0:
                        gt = zg_pool.tile([128, KB3 * D], bf16, tag="zgg")
                        nc.gpsimd.indirect_dma_start(
                            out=gt[:], out_offset=None, in_=ZT[:, :],
                            in_offset=IndirectOffsetOnAxis(
                                ap=esrcT[:, g * KB3:(g + 1) * KB3], axis=0))
                        globals()["_zbuf"] = gt
                    gt = globals()["_zbuf"]
                    eaT = work.tile([ED, 128], f32, tag="eaT")
                    nc.sync.dma_start(out=eaT[:],
                                      in_=eattrT_in[col * ED:(col + 1) * ED, :])
                    ef_ps = pswp.tile([128, D], f32, tag="psw")
                    nc.tensor.matmul(out=ef_ps[:], lhsT=eaT[:], rhs=EPW[:],
                                     start=True, stop=True)
                    ef = work.tile([128, D], f32, tag="ef")
                    if use_epb:
                        nc.vector.tensor_add(out=ef_ps[:], in0=ef_ps[:], in1=EPB[:])
                    nc.scalar.activation(out=ef[:], in_=ef_ps[:], func=AF.Relu)
                    sel = selp.tile([128, 128], f32, tag="sel")
                    nc.vector.tensor_tensor(
                        out=sel[:], in0=edstT[:, col:col + 1].to_broadcast([128, 128]),
                        in1=IOTA[:], op=OP.is_equal)
                    selb = selp.tile([128, 128], bf16, tag="selb")
                    nc.vector.tensor_copy(out=selb[:], in_=sel[:])
                    cc = col % KB3
                    nc.tensor.matmul(out=psA[:], lhsT=selb[:],
                                     rhs=gt[:, cc * D:(cc + 1) * D],
                                     start=(t == 0), stop=(t == T[b] - 1))
                    nc.tensor.matmul(out=psB[:], lhsT=sel[:], rhs=ef[:],
                                     start=(t == 0), stop=(t == T[b] - 1))
                o1 = work.tile([128, D], f32, tag="o1")
                nc.vector.tensor_scalar_mul(out=o1[:], in0=psA[:],
                                            scalar1=ASC[:, b:b + 1])
                o2 = work.tile([128, D], f32, tag="o2")
                nc.vector.tensor_scalar_mul(out=o2[:],
                                            in0=XW[:, b * D:(b + 1) * D],
                                            scalar1=DW2[:, b:b + 1])
                nc.vector.tensor_add(out=o1[:], in0=o1[:], in1=psB[:])
                nc.vector.tensor_add(out=o1[:], in0=o1[:], in1=o2[:])
                if use_convb:
                    nc.vector.tensor_add(out=o1[:], in0=o1[:], in1=CONVB[:])
                ot = work.tile([128, D], f32, tag="ot")
                nc.scalar.activation(out=ot[:], in_=o1[:], func=AF.Relu)
                nc.sync.dma_start(out=out_p[b * 128:(b + 1) * 128, :], in_=ot[:])

    nc.compile()
    return nc


def kernel(x, edge_index, edge_attr, u_in, u_out, ln_g, ln_b, conv_w, conv_b,
           ep_w, ep_b, ia1_w, ia1_b, ia2_w, ia2_b, oa1_w, oa1_b, oa2_w, oa2_b):
    global LAST_EXEC_NS, LAST_RESULTS
    from concourse.bass_utils import run_bass_kernel_spmd

    x = np.asarray(x, np.float32)
    edge_index = np.asarray(edge_index, np.int32)
    edge_attr = np.asarray(edge_attr, np.float32)
    u_in = np.asarray(u_in, np.float32)
    u_out = np.asarray(u_out, np.float32)

    prep = _host_prep(x, edge_index, edge_attr, u_in, u_out)

    W1cat = np.concatenate([ia1_w, oa1_w], axis=1).astype(np.float32)
    W2blk = np.zeros((2 * H, 4), np.float32)
    W2blk[:H, :2] = ia2_w
    W2blk[H:, 2:] = oa2_w
    b1cat = np.concatenate([ia1_b, oa1_b]).astype(np.float32)
    b2cat = np.concatenate([ia2_b, oa2_b]).astype(np.float32)
    use_b1 = bool(np.any(b1cat))
    use_b2 = bool(np.any(b2cat))
    use_epb = bool(np.any(np.asarray(ep_b)))
    use_convb = bool(np.any(np.asarray(conv_b)))

    key = (prep["T"], prep["NTP"], use_b1, use_b2, use_epb, use_convb)
    if key not in _compiled:
        _compiled[key] = _build(prep["T"], prep["NTP"],
                                use_b1, use_b2, use_epb, use_convb)
    nc = _compiled[key]

    common = dict(
        W1cat=W1cat, W2blk=W2blk,
        conv_w=np.asarray(conv_w, np.float32),
        ep_w=np.asarray(ep_w, np.float32),
        ln_g=np.asarray(ln_g, np.float32).reshape(128, 1),
        ln_b=np.asarray(ln_b, np.float32).reshape(128, 1),
        IOTA=np.tile(np.arange(128, dtype=np.float32), (128, 1)),
        IDENT=np.eye(128, dtype=np.float32),
    )
    if use_b1:
        common["B1REP"] = np.tile(b1cat, (128, 1))
    if use_b2:
        common["B2REP"] = np.tile(b2cat, (128, 1))
    if use_epb:
        common["EPBREP"] = np.tile(np.asarray(ep_b, np.float32), (128, 1))
    if use_convb:
        common["CONVBREP"] = np.tile(np.asarray(conv_b, np.float32), (128, 1))

    in_maps = []
    for c in range(NCORES):
        m = dict(common)
        m["xs"] = prep["xs"][c]
        m["esrcT"] = prep["esrcT"][c]
        m["edstT"] = prep["edstT"][c]
        m["eattrT"] = prep["eattrT"][c]
        m["deg1"] = prep["deg1"][c]
        m["u_in"] = prep["u_in"][c]
        m["u_out"] = prep["u_out"][c]
        in_maps.append(m)

    import time as _time
    trace = bool(os.environ.get("KERNEL_TRACE"))
    t0 = _time.time()
    try:
        res = run_bass_kernel_spmd(nc, in_maps, list(range(NCORES)), trace=trace)
    except ModuleNotFoundError:
        res = run_bass_kernel_spmd(nc, in_maps, list(range(NCORES)), trace=False)
    globals()["LAST_WALL_NS"] = int((_time.time() - t0) * 1e9)
    LAST_EXEC_NS = res.exec_time_ns
    LAST_RESULTS = res

    out = np.empty((N, D), np.float32)
    for c in range(NCORES):
        out[c * CHUNK:(c + 1) * CHUNK] = res.results[c]["out"][:CHUNK]
    return out

